# revision 76
# baseline (speedup 1.0000x reference)
"""GQA attention kernel for Trainium2 (8 NeuronCores).

Sharding: core = b*4 + g  (b = batch 0..1, g = kv-group 0..3).
Each core handles one batch element and one kv head (4 query heads),
computes q/k/v projections + RoPE + causal attention + a partial o_proj
(columns of Wo for its 4 heads). Host sums the 4 partials per batch.

All on-chip operands are bf16 (PSUM accumulation stays f32): halves DMA
traffic and doubles DVE elementwise throughput. Single fused pipeline:
projection chunks, RoPE, attention tiles and o_proj are interleaved in
one instruction stream so the PE never idles long enough to drop out of
its fast p-state.

Layouts on chip:
  xT   [D=1024, S=2048]  as 8 tiles [128, S]   (host-pretransposed x[b].T)
  w    [128, 8*384]      packed per-e weight slices (q0q1 | q2q3 | k,v)
  qT   [128 (2 heads x 64d), S] x 2 tiles (transposed, RoPE'd, bf16)
  kT   [128, S]          (64-row k duplicated into both halves)
  vn   16 tiles [128, 65] (v natural + ones column for softmax denom)
  scoresT[ki, qi] = k @ q^T -> exp -> 0/1 mask mul (diagonal tiles) ->
  ot_aug = vn^T @ attnT   ([65, 1024]: rows 0..63 = out^T per 2 heads,
  row 64 = softmax denominator).  Normalize on Pool engine, o_proj from
  bf16 oT, y written as bf16 and summed on host.

Scheduling: one fused stream; chunk-1 projections, RoPE and o_proj tiles
are queued as "fillers" pumped between attention tiles so the PE stays
continuously busy (p-state stays at 2.4 GHz); AV matmuls lag scores by 3
tiles to hide the exp dependency; the final chunk's normalization is
split into st-aligned column pieces so the tail o_proj pipelines.
"""

import numpy as np

B, S, D = 2, 2048, 1024
NH, NKV, HD = 16, 4, 64
SCALE = 1.0 / 8.0
ROPE_BASE = 10000.0

SC = 512  # attention q-chunk
NCH = S // SC  # 4
PC = 1024  # projection q-chunk

LAST_RESULT = None
LAST_IN_MAPS = None
_PROG = None


def _build_program():
    from contextlib import ExitStack

    import concourse.bass as bass  # noqa: F401
    import concourse.tile as tile
    from concourse import bacc, mybir

    f32 = mybir.dt.float32
    bf16 = mybir.dt.bfloat16
    EXP = mybir.ActivationFunctionType.Exp

    nc = bacc.Bacc(trn_type="TRN2")

    xT_d = nc.dram_tensor("xT", [D, S], bf16, kind="ExternalInput")
    w_d = nc.dram_tensor("wp", [128, 8 * 384], bf16, kind="ExternalInput")
    cs_d = nc.dram_tensor("cs", [128, 2 * S], bf16, kind="ExternalInput")
    cc_d = nc.dram_tensor("cc", [128, 192], bf16, kind="ExternalInput")
    wo_d = nc.dram_tensor("wop", [128, 2048], bf16, kind="ExternalInput")
    y_d = nc.dram_tensor("y", [S, D], bf16, kind="ExternalOutput")

    with tile.TileContext(nc) as tc, ExitStack() as ctx:
        const = ctx.enter_context(tc.tile_pool(name="const", bufs=1))
        pers = ctx.enter_context(tc.tile_pool(name="pers", bufs=1))
        mm = ctx.enter_context(tc.tile_pool(name="mm", bufs=1, space="PSUM"))
        pss = ctx.enter_context(tc.tile_pool(name="pss", bufs=2, space="PSUM"))
        pot = ctx.enter_context(tc.tile_pool(name="pot", bufs=1, space="PSUM"))
        sq = ctx.enter_context(tc.tile_pool(name="sq", bufs=2))
        sat = ctx.enter_context(tc.tile_pool(name="sat", bufs=8))
        sos = ctx.enter_context(tc.tile_pool(name="sos", bufs=3))
        snr = ctx.enter_context(tc.tile_pool(name="snr", bufs=2))
        syt = ctx.enter_context(tc.tile_pool(name="syt", bufs=6))

        w_sb = const.tile([128, 8 * 384], bf16, tag="w")
        cs_sb = const.tile([128, 2 * S], bf16, tag="cs")
        cc_sb = const.tile([128, 192], bf16, tag="cc")
        wo_sb = const.tile([128, 2048], bf16, tag="wo")

        xT_sb = [pers.tile([128, S], bf16, tag=f"xT{e}", name=f"xT{e}") for e in range(8)]
        qT = [pers.tile([128, S], bf16, tag=f"qT{m}", name=f"qT{m}") for m in range(2)]
        kT = pers.tile([128, S], bf16, tag="kT")
        vn = [pers.tile([128, 65], bf16, tag=f"vn{t}", name=f"vn{t}") for t in range(16)]
        oT = [pers.tile([128, S], bf16, tag=f"oT{p}", name=f"oT{p}") for p in range(2)]

        # Spread input loads across engine DGE queues so transfers overlap.
        # Two waves: chunk-0 projections only touch xT[:, 0:1024] (and the
        # first cos/sin halves), so those load first; second halves follow
        # and are only needed once chunk-1 projections start.
        def xw(eng, e, lo, hi):
            eng.dma_start(xT_sb[e][:, lo:hi], xT_d[e * 128 : (e + 1) * 128, lo:hi])

        # keep the Activation queue nearly free: it must start casting and
        # exping as soon as the first projection lands
        nc.gpsimd.dma_start(w_sb[:, 0:768], w_d[:, 0:768])
        nc.scalar.dma_start(w_sb[:, 1536:3072], w_d[:, 1536:3072])
        nc.gpsimd.dma_start(w_sb[:, 768:1536], w_d[:, 768:1536])
        for e in (0, 1):
            xw(nc.sync, e, 0, 1024)
        for e in (4, 6):
            xw(nc.gpsimd, e, 0, 1024)
        xw(nc.sync, 5, 0, 1024)
        nc.gpsimd.dma_start(cs_sb[:, 0:1024], cs_d[:, 0:1024])
        nc.sync.dma_start(cs_sb[:, 2048:3072], cs_d[:, 2048:3072])
        xw(nc.gpsimd, 7, 0, 1024)
        xw(nc.sync, 2, 0, 1024)
        xw(nc.gpsimd, 3, 0, 1024)
        nc.sync.dma_start(cc_sb, cc_d[:, :])
        # wave B: second halves
        for e in (0, 1, 6):
            xw(nc.sync, e, 1024, 2048)
        for e in (4, 5, 7):
            xw(nc.gpsimd, e, 1024, 2048)
        xw(nc.sync, 2, 1024, 2048)
        nc.gpsimd.dma_start(cs_sb[:, 1024:2048], cs_d[:, 1024:2048])
        nc.gpsimd.dma_start(cs_sb[:, 3072:4096], cs_d[:, 3072:4096])
        xw(nc.sync, 3, 1024, 2048)
        nc.sync.dma_start(wo_sb, wo_d[:, :])

        cos_ap = cs_sb[:, 0:S]
        sin_ap = cs_sb[:, S : 2 * S]
        mask_ap = cc_sb[:, 0:128]
        id_ap = cc_sb[64:128, 128:192]

        for t in range(16):
            nc.gpsimd.memset(vn[t][:, 64:65], 1.0)

        # warm the Exp activation table during the input-DMA window
        warm = sq.tile([1, 8], f32, tag="warm")
        nc.vector.memset(warm, 0.0)
        nc.scalar.activation(warm, warm, EXP, scale=1.0)

        # ---------- emission helpers --------------------------------------
        def emit_proj_mm(C, m, ps, e, first=None, last=None):
            # one (e) step of the m-th projection for PC-chunk C: 2 matmuls
            # m in {0,1}: 128 q rows; m == 2: 64 k rows + 64 v rows
            nrow = 128
            for h2 in range(2):
                nc.tensor.matmul(
                    ps[0:nrow, h2 * 512 : (h2 + 1) * 512],
                    w_sb[:, e * 384 + m * 128 : e * 384 + m * 128 + nrow],
                    xT_sb[e][:, C * PC + h2 * 512 : C * PC + (h2 + 1) * 512],
                    start=(e == 0) if first is None else first,
                    stop=(e == 7) if last is None else last,
                    skip_group_check=True,
                )

        def emit_rope_q(C, m, ps, cast_dve=False, halves=False):
            qb = sq.tile([128, PC], bf16, tag="qb", name=f"qb{C}{m}")
            if cast_dve:
                nc.vector.tensor_copy(qb, ps)
            else:
                nc.scalar.copy(qb, ps)
            tm = sq.tile([128, PC], bf16, tag="tm", name=f"tm{C}{m}")
            ranges = [(0, 512), (512, PC)] if halves else [(0, PC)]
            for lo, hi in ranges:
                cslice = slice(C * PC + lo, C * PC + hi)
                for b0 in (0, 64):
                    nc.vector.tensor_copy(tm[b0 : b0 + 32, lo:hi],
                                          qb[b0 + 32 : b0 + 64, lo:hi])
                    nc.vector.tensor_copy(tm[b0 + 32 : b0 + 64, lo:hi],
                                          qb[b0 : b0 + 32, lo:hi])
                nc.vector.tensor_mul(tm[:, lo:hi], tm[:, lo:hi],
                                     sin_ap[:, cslice])
                nc.vector.tensor_mul(qb[:, lo:hi], qb[:, lo:hi],
                                     cos_ap[:, cslice])
                nc.vector.tensor_add(qT[m][:, cslice], qb[:, lo:hi],
                                     tm[:, lo:hi])

        def emit_rope_kv(C, ps, cast_dve=False, halves=False):
            # rows 0:64 of ps = k projection (RoPE + duplicate into both halves)
            kvb = sq.tile([128, PC], bf16, tag="qb", name=f"kvb{C}")
            if cast_dve:
                nc.vector.tensor_copy(kvb, ps)
            else:
                nc.scalar.copy(kvb, ps)
            kvb_box[C] = kvb
            tm = sq.tile([128, PC], bf16, tag="tm", name=f"tmk{C}")
            ranges = [(0, 512), (512, PC)] if halves else [(0, PC)]
            for lo, hi in ranges:
                cslice = slice(C * PC + lo, C * PC + hi)
                nc.vector.tensor_copy(tm[0:32, lo:hi], kvb[32:64, lo:hi])
                nc.vector.tensor_copy(tm[32:64, lo:hi], kvb[0:32, lo:hi])
                nc.vector.tensor_mul(tm[0:64, lo:hi], tm[0:64, lo:hi],
                                     sin_ap[0:64, cslice])
                nc.vector.tensor_mul(kvb[0:64, lo:hi], kvb[0:64, lo:hi],
                                     cos_ap[0:64, cslice])
                nc.vector.tensor_add(kT[0:64, cslice], kvb[0:64, lo:hi],
                                     tm[0:64, lo:hi])
                nc.vector.tensor_copy(kT[64:128, cslice], kT[0:64, cslice])

        kvb_box = {}

        def emit_vtrans(C, jjs):
            # transpose v rows of kvb into natural layout via the PE,
            # using a bf16 view of an f32 PSUM tile
            t8 = mm.tile([128, PC], f32, tag="mm", name=f"vt{C}{jjs[0]}")
            t8b = t8[:, :].bitcast(bf16)
            kvb = kvb_box[C]
            for jj in jjs:
                t = C * 8 + jj
                nc.tensor.transpose(
                    t8b[:, jj * 64 : (jj + 1) * 64],
                    kvb[64:128, jj * 128 : (jj + 1) * 128],
                    id_ap,
                )
                nc.vector.tensor_copy(vn[t][:, 0:64], t8b[:, jj * 64 : (jj + 1) * 64])

        def emit_oproj_half(st, psy, p2):
            for e2 in range(2):
                nc.tensor.matmul(
                    psy[:, e2 * 512 : (e2 + 1) * 512],
                    oT[p2][:, st * 128 : (st + 1) * 128],
                    wo_sb[:, p2 * 1024 + e2 * 512 : p2 * 1024 + (e2 + 1) * 512],
                    start=(p2 == 0),
                    stop=(p2 == 1),
                    skip_group_check=True,
                )

        def emit_oproj_st(st, tail=False):
            psy = mm.tile([128, 1024], f32, tag="mm", name=f"psy{st}")
            for p2 in range(2):
                emit_oproj_half(st, psy, p2)
            yt = syt.tile([128, 1024], bf16, tag="yt", name=f"yt{st}")
            if tail:
                # parallel half-casts on DVE + Act to shorten the critical tail
                nc.vector.tensor_copy(yt[:, 0:512], psy[:, 0:512])
                nc.scalar.copy(yt[:, 512:1024], psy[:, 512:1024])
            else:
                nc.vector.tensor_copy(yt, psy)
            nc.sync.dma_start(y_d[st * 128 : (st + 1) * 128, :], yt)

        # filler machinery: closures emitted between attention tiles
        fillers = []

        def pump(n=1):
            for _ in range(n):
                if fillers:
                    f = fillers.pop(0)
                    if f is not None:
                        f()

        # ---------- intro: projections + rope for PC-chunk 0 --------------
        # e consumed in approximate DMA-arrival order; m0/m1 use the (still
        # idle) scores pool so the three projections don't serialize on mm
        E_ORDER = [0, 4, 1, 6, 5, 2, 7, 3]
        for m in (2, 0, 1):
            pool, tg = (pss, "ps") if m < 2 else (mm, "mm")
            ps = pool.tile([128, PC], f32, tag=tg, name=f"pj0{m}")
            for i, e in enumerate(E_ORDER):
                emit_proj_mm(0, m, ps, e, first=(i == 0), last=(i == 7))
            if m == 2:
                emit_rope_kv(0, ps, halves=True)
            else:
                emit_rope_q(0, m, ps, halves=True)
                if m == 0:
                    emit_vtrans(0, list(range(8)))

        # each proj filler does 2 e's (4 matmuls); fin does the rope
        def mk_proj2_steps(C, m, ps_box):
            def step(e0, C=C, m=m):
                def f():
                    if "ps" not in ps_box:
                        ps_box["ps"] = mm.tile(
                            [128, PC], f32, tag="mm", name=f"pj{C}{m}"
                        )
                    emit_proj_mm(C, m, ps_box["ps"], e0)
                    emit_proj_mm(C, m, ps_box["ps"], e0 + 1)

                return f

            return [step(e0) for e0 in range(0, 8, 2)]

        def mk_proj2_fin(C, m, ps_box):
            def f():
                if m < 2:
                    emit_rope_q(C, m, ps_box["ps"])
                else:
                    emit_rope_kv(C, ps_box["ps"])

            return [f]

        def mk_proj2(C, m):
            ps_box = {}
            return mk_proj2_steps(C, m, ps_box) + mk_proj2_fin(C, m, ps_box)

        # ---------- main loop over attention chunks ------------------------
        for c in range(NCH):
            if c == 1:
                for m in range(3):
                    for f in mk_proj2(1, m):
                        fillers.append(f)
                        fillers.append(None)
                fillers.append(lambda: emit_vtrans(1, [0, 1, 2, 3]))
                fillers.append(lambda: emit_vtrans(1, [4, 5, 6, 7]))
            if c == 2:
                # correctness deadline: chunk-1 proj/rope must be emitted
                while fillers:
                    pump(1)
                for st in range(0, 8):
                    fillers.append(lambda st=st: emit_oproj_st(st))
                    fillers.extend([None, None, None, None])
            if c == 3:
                for st in range(8, 12):
                    fillers.append(lambda st=st: emit_oproj_st(st))
                    fillers.extend([None] * 9)

            for p in range(2):
                nt = 4 * c + 4
                otile = pot.tile([65, 1024], f32, tag="ot", name=f"ot{c}{p}")
                def emit_av(t, at, off):
                    for hh in range(2):
                        nc.tensor.matmul(
                            otile[:, hh * 512 + off : (hh + 1) * 512],
                            vn[t][:, 0:65],
                            at[:, hh * 512 + off : (hh + 1) * 512],
                            start=(t == 0),
                            stop=(t == nt - 1),
                            skip_group_check=True,
                        )

                pends = []  # AV lags scores to hide exp latency
                lag = 4
                for t in range(nt):
                    j = t - 4 * c
                    off = 128 * j if j > 0 else 0
                    ps = pss.tile([128, 1024], f32, tag="ps", name=f"ps{c}{p}{t}")
                    for hh in range(2):
                        nc.tensor.matmul(
                            ps[:, hh * 512 + off : (hh + 1) * 512],
                            kT[hh * 64 : (hh + 1) * 64, t * 128 : (t + 1) * 128],
                            qT[p][hh * 64 : (hh + 1) * 64,
                                  c * SC + off : (c + 1) * SC],
                            start=True,
                            stop=True,
                        )
                    at = sat.tile([128, 1024], bf16, tag="at", name=f"at{c}{p}{t}")
                    if off == 0:
                        nc.scalar.activation(at, ps, EXP, scale=SCALE)
                    else:
                        pin = ps[:, :].rearrange("p (b w) -> p b w", b=2)[:, :, off:512]
                        pat = at[:, :].rearrange("p (b w) -> p b w", b=2)[:, :, off:512]
                        nc.scalar.activation(pat, pin, EXP, scale=SCALE)
                    if j >= 0:
                        ab = at[:, :].rearrange("p (b w) -> p b w", b=2)[
                            :, :, off : off + 128]
                        mb = mask_ap.unsqueeze(1).broadcast_to([128, 2, 128])
                        nc.vector.tensor_mul(ab, ab, mb)
                    pends.append((t, at, off))
                    if len(pends) > lag:
                        emit_av(*pends.pop(0))
                    pump(1)
                for pe_ in pends:
                    emit_av(*pe_)
                if c == 3 and p == 1:
                    break  # normalization handled in the tail below
                # normalization: 1/denom on DVE, broadcast + muls on Pool
                osb = sos.tile([65, 1024], f32, tag="osb", name=f"os{c}{p}")
                nc.vector.tensor_copy(osb, otile)
                rr = snr.tile([1, 1024], f32, tag="rr", name=f"rr{c}{p}")
                nc.vector.reciprocal(rr, osb[64:65, :])
                rb = snr.tile([64, 1024], f32, tag="rb", name=f"rb{c}{p}")
                nc.gpsimd.partition_broadcast(rb, rr)
                for hh in range(2):
                    nc.gpsimd.tensor_mul(
                        oT[p][hh * 64 : (hh + 1) * 64, c * SC : (c + 1) * SC],
                        osb[0:64, hh * 512 : (hh + 1) * 512],
                        rb[:, hh * 512 : (hh + 1) * 512],
                    )

        # ---------- tail: last norm split by column-halves + o_proj --------
        while fillers:
            pump(1)
        # chunk-3 columns normalized in 4 st-aligned pieces; each piece
        # unblocks one o_proj output tile so the tail pipelines
        psy_t = {
            12: pss.tile([128, 1024], f32, tag="ps", name="psy12"),
            13: pss.tile([128, 1024], f32, tag="ps", name="psy13"),
            14: mm.tile([128, 1024], f32, tag="mm", name="psy14"),
        }
        for st in (12, 13, 14):
            emit_oproj_half(st, psy_t[st], 0)
        osb = sos.tile([65, 1024], f32, tag="osb", name="os31")
        rr = snr.tile([1, 1024], f32, tag="rr", name="rr31")
        rb = snr.tile([64, 1024], f32, tag="rb", name="rb31")

        def norm_piece(q):
            # columns [128q, 128q+128) of both hh blocks, via 3-dim APs;
            # copies alternate Act/DVE so the two chains pipeline
            def blk(ap2d):
                return ap2d.rearrange("p (b w) -> p b w", b=2)[
                    :, :, 128 * q : 128 * q + 128]
            if q % 2 == 0:
                nc.scalar.copy(blk(osb[:, :]), blk(otile[:, :]))
            else:
                nc.vector.tensor_copy(blk(osb[:, :]), blk(otile[:, :]))
            nc.vector.reciprocal(blk(rr[:, :]), blk(osb[64:65, :]))
            nc.gpsimd.partition_broadcast(blk(rb[:, :]), blk(rr[:, :]))
            for hh in range(2):
                cs0 = slice(hh * 512 + 128 * q, hh * 512 + 128 * q + 128)
                nc.gpsimd.tensor_mul(
                    oT[1][hh * 64 : (hh + 1) * 64,
                          3 * SC + 128 * q : 3 * SC + 128 * q + 128],
                    osb[0:64, cs0],
                    rb[:, cs0],
                )

        def emit_oproj_fin(st, psy):
            yt = syt.tile([128, 1024], bf16, tag="yt", name=f"yt{st}")
            nc.vector.tensor_copy(yt[:, 0:512], psy[:, 0:512])
            nc.scalar.copy(yt[:, 512:1024], psy[:, 512:1024])
            nc.sync.dma_start(y_d[st * 128 : (st + 1) * 128, :], yt)

        def emit_fin_act(st, psy):
            # tail casts split across DVE + Act halves (both idle by now);
            # y DMA in halves so each half ships as soon as its cast lands
            yt = syt.tile([128, 1024], bf16, tag="yt", name=f"yt{st}")
            nc.scalar.copy(yt[:, 0:512], psy[:, 0:512])
            nc.vector.tensor_copy(yt[:, 512:1024], psy[:, 512:1024])
            nc.sync.dma_start(y_d[st * 128 : (st + 1) * 128, :], yt)

        psy15 = None
        for q, st in enumerate((12, 13, 14)):
            norm_piece(q)
            emit_oproj_half(st, psy_t[st], 1)
            emit_fin_act(st, psy_t[st])
            if st == 12:
                psy15 = pss.tile([128, 1024], f32, tag="ps", name="psy15")
                emit_oproj_half(15, psy15, 0)
        norm_piece(3)
        emit_oproj_half(15, psy15, 1)
        emit_fin_act(15, psy15)

    nc.compile()
    return nc


def _host_constants(np_bf16):
    inv = 1.0 / (ROPE_BASE ** (np.arange(0, HD, 2, dtype=np.float64) / HD))
    freqs = np.outer(np.arange(S, dtype=np.float64), inv)  # [S, 32]
    emb = np.concatenate([freqs, freqs], axis=-1)  # [S, 64]
    cos = np.cos(emb).astype(np.float32).T  # [64, S]
    sin = np.sin(emb).astype(np.float32).T
    sgn = np.concatenate([-np.ones((32, 1)), np.ones((32, 1))]).astype(np.float32)
    sin_signed = sin * sgn
    cos128 = np.concatenate([cos, cos], axis=0)
    sin128 = np.concatenate([sin_signed, sin_signed], axis=0)
    cs = np.ascontiguousarray(
        np.concatenate([cos128, sin128], axis=1)
    ).astype(np_bf16)  # [128, 2S]
    ki = np.arange(128)[:, None]
    qi = np.arange(128)[None, :]
    maskb = (ki <= qi).astype(np.float32)  # keep lower incl diag (ki <= qi)
    cc = np.zeros((128, 192), dtype=np.float32)
    cc[:, 0:128] = maskb
    cc[64:128, 128:192] = np.eye(64, dtype=np.float32)
    return cs, cc.astype(np_bf16)


def kernel(x, Wq, Wk, Wv, Wo):
    global LAST_RESULT, _PROG
    from concourse import bass_utils, mybir

    np_bf16 = mybir.dt.np(mybir.dt.bfloat16)

    x = np.asarray(x, dtype=np.float32)
    Wq = np.asarray(Wq, dtype=np.float32)
    Wk = np.asarray(Wk, dtype=np.float32)
    Wv = np.asarray(Wv, dtype=np.float32)
    Wo = np.asarray(Wo, dtype=np.float32)

    if _PROG is None:
        _PROG = _build_program()
    nc = _PROG

    cs, cc = _host_constants(np_bf16)
    WoT = np.ascontiguousarray(Wo.T)  # [c, e]
    Wqh = Wq.reshape(NH, HD, D)
    Wkh = Wk.reshape(NKV, HD, D)
    Wvh = Wv.reshape(NKV, HD, D)

    in_maps = []
    for core in range(8):
        b, g = core // 4, core % 4
        xT = np.ascontiguousarray(x[b].T).astype(np_bf16)
        wcat = np.concatenate(
            [Wqh[4 * g : 4 * g + 4].reshape(4 * HD, D), Wkh[g], Wvh[g]], axis=0
        )  # [384, D]
        # pack [D, 384] -> [128, 8*384] (per 128-row e-slice side by side)
        wp = (
            np.ascontiguousarray(wcat.T)
            .reshape(8, 128, 384)
            .transpose(1, 0, 2)
            .reshape(128, 8 * 384)
        ).astype(np_bf16)
        wop = (
            WoT[g * 256 : (g + 1) * 256, :]
            .reshape(2, 128, D)
            .transpose(1, 0, 2)
            .reshape(128, 2 * D)
        ).astype(np_bf16)
        in_maps.append(
            {
                "xT": np.ascontiguousarray(xT),
                "wp": np.ascontiguousarray(wp),
                "cs": cs,
                "cc": cc,
                "wop": np.ascontiguousarray(wop),
            }
        )

    global LAST_IN_MAPS
    LAST_IN_MAPS = in_maps
    res = bass_utils.run_bass_kernel_spmd(nc, in_maps, core_ids=list(range(8)))
    LAST_RESULT = res
    ys = [np.asarray(m["y"]).astype(np.float32) for m in res.results]
    out = np.stack(
        [ys[0] + ys[1] + ys[2] + ys[3], ys[4] + ys[5] + ys[6] + ys[7]], axis=0
    )
    return out


def benchmark(n_iters=50):
    """Estimate steady-state per-execution device time of the NEFF.

    Dispatches the jitted bass_exec N times asynchronously and blocks once
    at the end; reports (T(N2)-T(N1))/(N2-N1) to cancel fixed dispatch /
    transfer overhead.
    """
    import time

    import jax
    import numpy as np
    from jax.experimental.shard_map import shard_map
    from jax.sharding import Mesh, NamedSharding, PartitionSpec

    import concourse.mybir as mybir
    from concourse.bass2jax import (
        _bass_exec_p,
        install_neuronx_cc_hook,
        partition_id_tensor,
    )

    assert _PROG is not None and LAST_IN_MAPS is not None, "run kernel() first"
    nc = _PROG
    in_maps = LAST_IN_MAPS
    n_cores = 8

    install_neuronx_cc_hook()
    partition_name = nc.partition_id_tensor.name if nc.partition_id_tensor else None
    in_names, out_names, out_avals, zero_outs = [], [], [], []
    for alloc in nc.m.functions[0].allocations:
        if not isinstance(alloc, mybir.MemoryLocationSet):
            continue
        name = alloc.memorylocations[0].name
        if alloc.kind == "ExternalInput":
            if name != partition_name:
                in_names.append(name)
        elif alloc.kind == "ExternalOutput":
            dt = mybir.dt.np(alloc.dtype)
            out_avals.append(jax.core.ShapedArray(tuple(alloc.tensor_shape), dt))
            out_names.append(name)
            zero_outs.append(np.zeros(tuple(alloc.tensor_shape), dt))
    n_params = len(in_names)
    # full operand-name list: inputs, then donated output slots, then
    # partition id — must match run_bass_via_pjrt's convention.
    in_names_full = list(in_names) + list(out_names)
    if partition_name is not None:
        in_names_full.append(partition_name)

    def _body(*args):
        operands = list(args)
        if partition_name is not None:
            operands.append(partition_id_tensor())
        outs = _bass_exec_p.bind(
            *operands,
            out_avals=tuple(out_avals),
            in_names=tuple(in_names_full),
            out_names=tuple(out_names),
            lowering_input_output_aliases=(),
            sim_require_finite=True,
            sim_require_nnan=True,
            nc=nc,
        )
        return tuple(outs)

    devices = jax.devices()[:n_cores]
    mesh = Mesh(np.asarray(devices), ("core",))
    n_outs = len(out_names)
    in_specs = (PartitionSpec("core"),) * (n_params + n_outs)
    out_specs = (PartitionSpec("core"),) * n_outs
    donate = tuple(range(n_params, n_params + n_outs))
    fn = jax.jit(
        shard_map(_body, mesh=mesh, in_specs=in_specs, out_specs=out_specs,
                  check_rep=False),
        donate_argnums=donate,
        keep_unused=True,
    )
    per_core = [[np.asarray(m[name]) for name in in_names] for m in in_maps]
    concat_in = [
        np.concatenate([per_core[c][i] for c in range(n_cores)], axis=0)
        for i in range(n_params)
    ]
    concat_zeros = [
        np.zeros((n_cores * z.shape[0], *z.shape[1:]), z.dtype) for z in zero_outs
    ]
    sh = NamedSharding(mesh, PartitionSpec("core"))
    params_dev = [jax.device_put(a, sh) for a in concat_in]
    z = [jax.device_put(a, sh) for a in concat_zeros]
    # warmup (compile + a few runs); chain outputs into donated slots
    for _ in range(3):
        outs = fn(*params_dev, *z)
        z = list(outs[:n_outs])
    jax.block_until_ready(z)

    def run(n):
        nonlocal z
        t0 = time.perf_counter()
        for _ in range(n):
            outs = fn(*params_dev, *z)
            z = list(outs[:n_outs])
        jax.block_until_ready(z)
        return time.perf_counter() - t0

    n1, n2 = max(5, n_iters // 5), n_iters
    t1 = run(n1)
    t2 = run(n2)
    per_iter = (t2 - t1) / (n2 - n1)
    print(f"benchmark: T({n1})={t1*1e3:.2f}ms T({n2})={t2*1e3:.2f}ms "
          f"slope={per_iter*1e6:.1f}us/iter")
    return per_iter


# revision 84
# speedup vs baseline: 1.6112x; 1.6112x over previous
"""GQA attention kernel for Trainium2 (8 NeuronCores).

Sharding: core = b*4 + g  (b = batch 0..1, g = kv-group 0..3).
Each core handles one batch element and one kv head (4 query heads),
computes q/k/v projections + RoPE + causal attention + a partial o_proj
(columns of Wo for its 4 heads). Host sums the 4 partials per batch.

All on-chip operands are bf16 (PSUM accumulation stays f32): halves DMA
traffic and doubles DVE elementwise throughput. Single fused pipeline:
projection chunks, RoPE, attention tiles and o_proj are interleaved in
one instruction stream so the PE never idles long enough to drop out of
its fast p-state.

Layouts on chip:
  xT   [D=1024, S=2048]  as 8 tiles [128, S]   (host-pretransposed x[b].T)
  w    [128, 8*384]      packed per-e weight slices (q0q1 | q2q3 | k,v)
  qT   [128 (2 heads x 64d), S] x 2 tiles (transposed, RoPE'd, bf16)
  kT   [128, S]          (64-row k duplicated into both halves)
  vn   16 tiles [128, 65] (v natural + ones column for softmax denom)
  scoresT[ki, qi] = k @ q^T -> exp -> 0/1 mask mul (diagonal tiles) ->
  ot_aug = vn^T @ attnT   ([65, 1024]: rows 0..63 = out^T per 2 heads,
  row 64 = softmax denominator).  Normalize on Pool engine, o_proj from
  bf16 oT, y written as bf16 and summed on host.

Scheduling: one fused stream; chunk-1 projections, RoPE and o_proj tiles
are queued as "fillers" pumped between attention tiles so the PE stays
continuously busy (p-state stays at 2.4 GHz); AV matmuls lag scores by 3
tiles to hide the exp dependency; the final chunk's normalization is
split into st-aligned column pieces so the tail o_proj pipelines.
"""

import numpy as np

B, S, D = 2, 2048, 1024
NH, NKV, HD = 16, 4, 64
SCALE = 1.0 / 8.0
ROPE_BASE = 10000.0

SC = 512  # attention q-chunk
NCH = S // SC  # 4
PC = 1024  # projection q-chunk

LAST_RESULT = None
LAST_IN_MAPS = None
_PROG = None


def _build_program():
    from contextlib import ExitStack

    import concourse.bass as bass  # noqa: F401
    import concourse.tile as tile
    from concourse import bacc, mybir

    f32 = mybir.dt.float32
    bf16 = mybir.dt.bfloat16
    EXP = mybir.ActivationFunctionType.Exp

    nc = bacc.Bacc(trn_type="TRN2")

    xT_d = nc.dram_tensor("xT", [D, S], bf16, kind="ExternalInput")
    w_d = nc.dram_tensor("wp", [128, 8 * 384], bf16, kind="ExternalInput")
    cs_d = nc.dram_tensor("cs", [128, 2 * S], bf16, kind="ExternalInput")
    cc_d = nc.dram_tensor("cc", [128, 192], bf16, kind="ExternalInput")
    wo_d = nc.dram_tensor("wop", [128, 2048], bf16, kind="ExternalInput")
    y_d = nc.dram_tensor("y", [S, D], bf16, kind="ExternalOutput")

    with tile.TileContext(nc) as tc, ExitStack() as ctx:
        const = ctx.enter_context(tc.tile_pool(name="const", bufs=1))
        pers = ctx.enter_context(tc.tile_pool(name="pers", bufs=1))
        mm = ctx.enter_context(tc.tile_pool(name="mm", bufs=1, space="PSUM"))
        pss = ctx.enter_context(tc.tile_pool(name="pss", bufs=2, space="PSUM"))
        pot = ctx.enter_context(tc.tile_pool(name="pot", bufs=1, space="PSUM"))
        sq = ctx.enter_context(tc.tile_pool(name="sq", bufs=2))
        sat = ctx.enter_context(tc.tile_pool(name="sat", bufs=8))
        sos = ctx.enter_context(tc.tile_pool(name="sos", bufs=3))
        snr = ctx.enter_context(tc.tile_pool(name="snr", bufs=2))
        syt = ctx.enter_context(tc.tile_pool(name="syt", bufs=6))

        w_sb = const.tile([128, 8 * 384], bf16, tag="w")
        cs_sb = const.tile([128, 2 * S], bf16, tag="cs")
        cc_sb = const.tile([128, 192], bf16, tag="cc")
        wo_sb = const.tile([128, 2048], bf16, tag="wo")

        xT_sb = [pers.tile([128, S], bf16, tag=f"xT{e}", name=f"xT{e}") for e in range(8)]
        qT = [pers.tile([128, S], bf16, tag=f"qT{m}", name=f"qT{m}") for m in range(2)]
        kT = pers.tile([128, S], bf16, tag="kT")
        vn = [pers.tile([128, 65], bf16, tag=f"vn{t}", name=f"vn{t}") for t in range(16)]
        oT = [pers.tile([128, S], bf16, tag=f"oT{p}", name=f"oT{p}") for p in range(2)]

        # Spread input loads across engine DGE queues so transfers overlap.
        # Two waves: chunk-0 projections only touch xT[:, 0:1024] (and the
        # first cos/sin halves), so those load first; second halves follow
        # and are only needed once chunk-1 projections start.
        def xw(eng, e, lo, hi):
            eng.dma_start(xT_sb[e][:, lo:hi], xT_d[e * 128 : (e + 1) * 128, lo:hi])

        # keep the Activation queue nearly free: it must start casting and
        # exping as soon as the first projection lands
        nc.gpsimd.dma_start(w_sb[:, 0:768], w_d[:, 0:768])
        nc.scalar.dma_start(w_sb[:, 1536:3072], w_d[:, 1536:3072])
        nc.gpsimd.dma_start(w_sb[:, 768:1536], w_d[:, 768:1536])
        for e in (0, 1):
            xw(nc.sync, e, 0, 1024)
        for e in (4, 6):
            xw(nc.gpsimd, e, 0, 1024)
        xw(nc.sync, 5, 0, 1024)
        nc.gpsimd.dma_start(cs_sb[:, 0:1024], cs_d[:, 0:1024])
        nc.sync.dma_start(cs_sb[:, 2048:3072], cs_d[:, 2048:3072])
        xw(nc.gpsimd, 7, 0, 1024)
        xw(nc.sync, 2, 0, 1024)
        xw(nc.gpsimd, 3, 0, 1024)
        nc.sync.dma_start(cc_sb, cc_d[:, :])
        # wave B: second halves
        for e in (0, 1, 6):
            xw(nc.sync, e, 1024, 2048)
        for e in (4, 5, 7):
            xw(nc.gpsimd, e, 1024, 2048)
        xw(nc.sync, 2, 1024, 2048)
        nc.gpsimd.dma_start(cs_sb[:, 1024:2048], cs_d[:, 1024:2048])
        nc.gpsimd.dma_start(cs_sb[:, 3072:4096], cs_d[:, 3072:4096])
        xw(nc.sync, 3, 1024, 2048)
        nc.sync.dma_start(wo_sb, wo_d[:, :])

        cos_ap = cs_sb[:, 0:S]
        sin_ap = cs_sb[:, S : 2 * S]
        mask_ap = cc_sb[:, 0:128]
        id_ap = cc_sb[64:128, 128:192]

        for t in range(16):
            nc.gpsimd.memset(vn[t][:, 64:65], 1.0)

        # warm the Exp activation table during the input-DMA window
        warm = sq.tile([1, 8], f32, tag="warm")
        nc.vector.memset(warm, 0.0)
        nc.scalar.activation(warm, warm, EXP, scale=1.0)

        # ---------- emission helpers --------------------------------------
        def emit_proj_mm(C, m, ps, e, first=None, last=None):
            # one (e) step of the m-th projection for PC-chunk C: 2 matmuls
            # m in {0,1}: 128 q rows; m == 2: 64 k rows + 64 v rows
            nrow = 128
            for h2 in range(2):
                nc.tensor.matmul(
                    ps[0:nrow, h2 * 512 : (h2 + 1) * 512],
                    w_sb[:, e * 384 + m * 128 : e * 384 + m * 128 + nrow],
                    xT_sb[e][:, C * PC + h2 * 512 : C * PC + (h2 + 1) * 512],
                    start=(e == 0) if first is None else first,
                    stop=(e == 7) if last is None else last,
                    skip_group_check=True,
                )

        def emit_rope_q(C, m, ps, cast_dve=False, halves=False):
            qb = sq.tile([128, PC], bf16, tag="qb", name=f"qb{C}{m}")
            if cast_dve:
                nc.vector.tensor_copy(qb, ps)
            else:
                nc.scalar.copy(qb, ps)
            tm = sq.tile([128, PC], bf16, tag="tm", name=f"tm{C}{m}")
            ranges = [(0, 512), (512, PC)] if halves else [(0, PC)]
            for lo, hi in ranges:
                cslice = slice(C * PC + lo, C * PC + hi)
                for b0 in (0, 64):
                    nc.vector.tensor_copy(tm[b0 : b0 + 32, lo:hi],
                                          qb[b0 + 32 : b0 + 64, lo:hi])
                    nc.vector.tensor_copy(tm[b0 + 32 : b0 + 64, lo:hi],
                                          qb[b0 : b0 + 32, lo:hi])
                nc.vector.tensor_mul(tm[:, lo:hi], tm[:, lo:hi],
                                     sin_ap[:, cslice])
                nc.vector.tensor_mul(qb[:, lo:hi], qb[:, lo:hi],
                                     cos_ap[:, cslice])
                nc.vector.tensor_add(qT[m][:, cslice], qb[:, lo:hi],
                                     tm[:, lo:hi])

        def emit_rope_kv(C, ps, cast_dve=False, halves=False):
            # rows 0:64 of ps = k projection (RoPE + duplicate into both halves)
            kvb = sq.tile([128, PC], bf16, tag="qb", name=f"kvb{C}")
            if cast_dve:
                nc.vector.tensor_copy(kvb, ps)
            else:
                nc.scalar.copy(kvb, ps)
            kvb_box[C] = kvb
            tm = sq.tile([128, PC], bf16, tag="tm", name=f"tmk{C}")
            ranges = [(0, 512), (512, PC)] if halves else [(0, PC)]
            for lo, hi in ranges:
                cslice = slice(C * PC + lo, C * PC + hi)
                nc.vector.tensor_copy(tm[0:32, lo:hi], kvb[32:64, lo:hi])
                nc.vector.tensor_copy(tm[32:64, lo:hi], kvb[0:32, lo:hi])
                nc.vector.tensor_mul(tm[0:64, lo:hi], tm[0:64, lo:hi],
                                     sin_ap[0:64, cslice])
                nc.vector.tensor_mul(kvb[0:64, lo:hi], kvb[0:64, lo:hi],
                                     cos_ap[0:64, cslice])
                nc.vector.tensor_add(kT[0:64, cslice], kvb[0:64, lo:hi],
                                     tm[0:64, lo:hi])
                nc.vector.tensor_copy(kT[64:128, cslice], kT[0:64, cslice])

        kvb_box = {}

        def emit_vtrans(C, jjs):
            # transpose v rows of kvb into natural layout via the PE,
            # using a bf16 view of an f32 PSUM tile
            t8 = mm.tile([128, PC], f32, tag="mm", name=f"vt{C}{jjs[0]}")
            t8b = t8[:, :].bitcast(bf16)
            kvb = kvb_box[C]
            for jj in jjs:
                t = C * 8 + jj
                nc.tensor.transpose(
                    t8b[:, jj * 64 : (jj + 1) * 64],
                    kvb[64:128, jj * 128 : (jj + 1) * 128],
                    id_ap,
                )
                nc.vector.tensor_copy(vn[t][:, 0:64], t8b[:, jj * 64 : (jj + 1) * 64])

        def emit_oproj_half(st, psy, p2):
            for e2 in range(2):
                nc.tensor.matmul(
                    psy[:, e2 * 512 : (e2 + 1) * 512],
                    oT[p2][:, st * 128 : (st + 1) * 128],
                    wo_sb[:, p2 * 1024 + e2 * 512 : p2 * 1024 + (e2 + 1) * 512],
                    start=(p2 == 0),
                    stop=(p2 == 1),
                    skip_group_check=True,
                )

        def emit_oproj_st(st, tail=False):
            psy = mm.tile([128, 1024], f32, tag="mm", name=f"psy{st}")
            for p2 in range(2):
                emit_oproj_half(st, psy, p2)
            yt = syt.tile([128, 1024], bf16, tag="yt", name=f"yt{st}")
            if tail:
                # parallel half-casts on DVE + Act to shorten the critical tail
                nc.vector.tensor_copy(yt[:, 0:512], psy[:, 0:512])
                nc.scalar.copy(yt[:, 512:1024], psy[:, 512:1024])
            else:
                nc.vector.tensor_copy(yt, psy)
            nc.sync.dma_start(y_d[st * 128 : (st + 1) * 128, :], yt)

        # filler machinery: closures emitted between attention tiles
        fillers = []

        def pump(n=1):
            for _ in range(n):
                if fillers:
                    f = fillers.pop(0)
                    if f is not None:
                        f()

        # ---------- intro: projections + rope for PC-chunk 0 --------------
        # e consumed in approximate DMA-arrival order; m0/m1 use the (still
        # idle) scores pool so the three projections don't serialize on mm
        E_ORDER = [0, 4, 1, 6, 5, 2, 7, 3]
        for m in (2, 0, 1):
            pool, tg = (pss, "ps") if m < 2 else (mm, "mm")
            ps = pool.tile([128, PC], f32, tag=tg, name=f"pj0{m}")
            for i, e in enumerate(E_ORDER):
                emit_proj_mm(0, m, ps, e, first=(i == 0), last=(i == 7))
            if m == 2:
                emit_rope_kv(0, ps, halves=True)
            else:
                emit_rope_q(0, m, ps, halves=True)
                if m == 0:
                    emit_vtrans(0, list(range(8)))

        # each proj filler does 2 e's (4 matmuls); fin does the rope
        def mk_proj2_steps(C, m, ps_box):
            def step(e0, C=C, m=m):
                def f():
                    if "ps" not in ps_box:
                        ps_box["ps"] = mm.tile(
                            [128, PC], f32, tag="mm", name=f"pj{C}{m}"
                        )
                    emit_proj_mm(C, m, ps_box["ps"], e0)
                    emit_proj_mm(C, m, ps_box["ps"], e0 + 1)

                return f

            return [step(e0) for e0 in range(0, 8, 2)]

        def mk_proj2_fin(C, m, ps_box):
            def f():
                if m < 2:
                    emit_rope_q(C, m, ps_box["ps"])
                else:
                    emit_rope_kv(C, ps_box["ps"])

            return [f]

        def mk_proj2(C, m):
            ps_box = {}
            return mk_proj2_steps(C, m, ps_box) + mk_proj2_fin(C, m, ps_box)

        # ---------- main loop over attention chunks ------------------------
        for c in range(NCH):
            if c == 1:
                for m in range(3):
                    for f in mk_proj2(1, m):
                        fillers.append(f)
                        fillers.append(None)
                fillers.append(lambda: emit_vtrans(1, [0, 1, 2, 3]))
                fillers.append(lambda: emit_vtrans(1, [4, 5, 6, 7]))
            if c == 2:
                # correctness deadline: chunk-1 proj/rope must be emitted
                while fillers:
                    pump(1)
                for st in range(0, 8):
                    fillers.append(lambda st=st: emit_oproj_st(st))
                    fillers.extend([None, None, None, None])
            if c == 3:
                for st in range(8, 12):
                    fillers.append(lambda st=st: emit_oproj_st(st))
                    fillers.extend([None] * 9)

            for p in range(2):
                nt = 4 * c + 4
                otile = pot.tile([65, 1024], f32, tag="ot", name=f"ot{c}{p}")
                def emit_av(t, at, off):
                    for hh in range(2):
                        nc.tensor.matmul(
                            otile[:, hh * 512 + off : (hh + 1) * 512],
                            vn[t][:, 0:65],
                            at[:, hh * 512 + off : (hh + 1) * 512],
                            start=(t == 0),
                            stop=(t == nt - 1),
                            skip_group_check=True,
                        )

                pends = []  # AV lags scores to hide exp latency
                lag = 4
                for t in range(nt):
                    j = t - 4 * c
                    off = 128 * j if j > 0 else 0
                    ps = pss.tile([128, 1024], f32, tag="ps", name=f"ps{c}{p}{t}")
                    for hh in range(2):
                        nc.tensor.matmul(
                            ps[:, hh * 512 + off : (hh + 1) * 512],
                            kT[hh * 64 : (hh + 1) * 64, t * 128 : (t + 1) * 128],
                            qT[p][hh * 64 : (hh + 1) * 64,
                                  c * SC + off : (c + 1) * SC],
                            start=True,
                            stop=True,
                        )
                    at = sat.tile([128, 1024], bf16, tag="at", name=f"at{c}{p}{t}")
                    if off == 0:
                        nc.scalar.activation(at, ps, EXP, scale=SCALE)
                    else:
                        pin = ps[:, :].rearrange("p (b w) -> p b w", b=2)[:, :, off:512]
                        pat = at[:, :].rearrange("p (b w) -> p b w", b=2)[:, :, off:512]
                        nc.scalar.activation(pat, pin, EXP, scale=SCALE)
                    if j >= 0:
                        ab = at[:, :].rearrange("p (b w) -> p b w", b=2)[
                            :, :, off : off + 128]
                        mb = mask_ap.unsqueeze(1).broadcast_to([128, 2, 128])
                        nc.vector.tensor_mul(ab, ab, mb)
                    pends.append((t, at, off))
                    if len(pends) > lag:
                        emit_av(*pends.pop(0))
                    pump(1)
                for pe_ in pends:
                    emit_av(*pe_)
                if c == 3 and p == 1:
                    break  # normalization handled in the tail below
                # normalization: 1/denom on DVE, broadcast + muls on Pool
                osb = sos.tile([65, 1024], f32, tag="osb", name=f"os{c}{p}")
                nc.vector.tensor_copy(osb, otile)
                rr = snr.tile([1, 1024], f32, tag="rr", name=f"rr{c}{p}")
                nc.vector.reciprocal(rr, osb[64:65, :])
                rb = snr.tile([64, 1024], f32, tag="rb", name=f"rb{c}{p}")
                nc.gpsimd.partition_broadcast(rb, rr)
                for hh in range(2):
                    nc.gpsimd.tensor_mul(
                        oT[p][hh * 64 : (hh + 1) * 64, c * SC : (c + 1) * SC],
                        osb[0:64, hh * 512 : (hh + 1) * 512],
                        rb[:, hh * 512 : (hh + 1) * 512],
                    )

        # ---------- tail: last norm split by column-halves + o_proj --------
        while fillers:
            pump(1)
        # chunk-3 columns normalized in 4 st-aligned pieces; each piece
        # unblocks one o_proj output tile so the tail pipelines
        psy_t = {
            12: pss.tile([128, 1024], f32, tag="ps", name="psy12"),
            13: pss.tile([128, 1024], f32, tag="ps", name="psy13"),
            14: mm.tile([128, 1024], f32, tag="mm", name="psy14"),
        }
        for st in (12, 13, 14):
            emit_oproj_half(st, psy_t[st], 0)
        osb = sos.tile([65, 1024], f32, tag="osb", name="os31")
        rr = snr.tile([1, 1024], f32, tag="rr", name="rr31")
        rb = snr.tile([64, 1024], f32, tag="rb", name="rb31")

        def norm_piece(q):
            # columns [128q, 128q+128) of both hh blocks, via 3-dim APs;
            # copies alternate Act/DVE so the two chains pipeline
            def blk(ap2d):
                return ap2d.rearrange("p (b w) -> p b w", b=2)[
                    :, :, 128 * q : 128 * q + 128]
            if q % 2 == 0:
                nc.scalar.copy(blk(osb[:, :]), blk(otile[:, :]))
            else:
                nc.vector.tensor_copy(blk(osb[:, :]), blk(otile[:, :]))
            nc.vector.reciprocal(blk(rr[:, :]), blk(osb[64:65, :]))
            nc.gpsimd.partition_broadcast(blk(rb[:, :]), blk(rr[:, :]))
            for hh in range(2):
                cs0 = slice(hh * 512 + 128 * q, hh * 512 + 128 * q + 128)
                nc.gpsimd.tensor_mul(
                    oT[1][hh * 64 : (hh + 1) * 64,
                          3 * SC + 128 * q : 3 * SC + 128 * q + 128],
                    osb[0:64, cs0],
                    rb[:, cs0],
                )

        def emit_oproj_fin(st, psy):
            yt = syt.tile([128, 1024], bf16, tag="yt", name=f"yt{st}")
            nc.vector.tensor_copy(yt[:, 0:512], psy[:, 0:512])
            nc.scalar.copy(yt[:, 512:1024], psy[:, 512:1024])
            nc.sync.dma_start(y_d[st * 128 : (st + 1) * 128, :], yt)

        def emit_fin_act(st, psy):
            # tail casts split across DVE + Act halves (both idle by now);
            # y DMA in halves so each half ships as soon as its cast lands
            yt = syt.tile([128, 1024], bf16, tag="yt", name=f"yt{st}")
            nc.scalar.copy(yt[:, 0:512], psy[:, 0:512])
            nc.vector.tensor_copy(yt[:, 512:1024], psy[:, 512:1024])
            nc.sync.dma_start(y_d[st * 128 : (st + 1) * 128, :], yt)

        psy15 = None
        for q, st in enumerate((12, 13, 14)):
            norm_piece(q)
            emit_oproj_half(st, psy_t[st], 1)
            emit_fin_act(st, psy_t[st])
            if st == 12:
                psy15 = pss.tile([128, 1024], f32, tag="ps", name="psy15")
                emit_oproj_half(15, psy15, 0)
        norm_piece(3)
        emit_oproj_half(15, psy15, 1)
        emit_fin_act(15, psy15)

    nc.compile()
    return nc


def _host_constants(np_bf16):
    inv = 1.0 / (ROPE_BASE ** (np.arange(0, HD, 2, dtype=np.float64) / HD))
    freqs = np.outer(np.arange(S, dtype=np.float64), inv)  # [S, 32]
    emb = np.concatenate([freqs, freqs], axis=-1)  # [S, 64]
    cos = np.cos(emb).astype(np.float32).T  # [64, S]
    sin = np.sin(emb).astype(np.float32).T
    sgn = np.concatenate([-np.ones((32, 1)), np.ones((32, 1))]).astype(np.float32)
    sin_signed = sin * sgn
    cos128 = np.concatenate([cos, cos], axis=0)
    sin128 = np.concatenate([sin_signed, sin_signed], axis=0)
    cs = np.ascontiguousarray(
        np.concatenate([cos128, sin128], axis=1)
    ).astype(np_bf16)  # [128, 2S]
    ki = np.arange(128)[:, None]
    qi = np.arange(128)[None, :]
    maskb = (ki <= qi).astype(np.float32)  # keep lower incl diag (ki <= qi)
    cc = np.zeros((128, 192), dtype=np.float32)
    cc[:, 0:128] = maskb
    cc[64:128, 128:192] = np.eye(64, dtype=np.float32)
    return cs, cc.astype(np_bf16)


def kernel(x, Wq, Wk, Wv, Wo):
    global LAST_RESULT, _PROG
    from concourse import bass_utils, mybir

    np_bf16 = mybir.dt.np(mybir.dt.bfloat16)

    x = np.asarray(x, dtype=np.float32)
    Wq = np.asarray(Wq, dtype=np.float32)
    Wk = np.asarray(Wk, dtype=np.float32)
    Wv = np.asarray(Wv, dtype=np.float32)
    Wo = np.asarray(Wo, dtype=np.float32)

    if _PROG is None:
        _PROG = _build_program()
    nc = _PROG

    cs, cc = _host_constants(np_bf16)
    WoT = np.ascontiguousarray(Wo.T)  # [c, e]
    Wqh = Wq.reshape(NH, HD, D)
    Wkh = Wk.reshape(NKV, HD, D)
    Wvh = Wv.reshape(NKV, HD, D)

    in_maps = []
    for core in range(8):
        b, g = core // 4, core % 4
        xT = np.ascontiguousarray(x[b].T).astype(np_bf16)
        wcat = np.concatenate(
            [Wqh[4 * g : 4 * g + 4].reshape(4 * HD, D), Wkh[g], Wvh[g]], axis=0
        )  # [384, D]
        # pack [D, 384] -> [128, 8*384] (per 128-row e-slice side by side)
        wp = (
            np.ascontiguousarray(wcat.T)
            .reshape(8, 128, 384)
            .transpose(1, 0, 2)
            .reshape(128, 8 * 384)
        ).astype(np_bf16)
        wop = (
            WoT[g * 256 : (g + 1) * 256, :]
            .reshape(2, 128, D)
            .transpose(1, 0, 2)
            .reshape(128, 2 * D)
        ).astype(np_bf16)
        in_maps.append(
            {
                "xT": np.ascontiguousarray(xT),
                "wp": np.ascontiguousarray(wp),
                "cs": cs,
                "cc": cc,
                "wop": np.ascontiguousarray(wop),
            }
        )

    global LAST_IN_MAPS
    LAST_IN_MAPS = in_maps
    res = bass_utils.run_bass_kernel_spmd(nc, in_maps, core_ids=list(range(8)))
    LAST_RESULT = res
    ys = [np.asarray(m["y"]).astype(np.float32) for m in res.results]
    out = np.stack(
        [ys[0] + ys[1] + ys[2] + ys[3], ys[4] + ys[5] + ys[6] + ys[7]], axis=0
    )
    return out


def benchmark(n_iters=50):
    """Estimate steady-state per-execution device time of the NEFF.

    Dispatches the jitted bass_exec N times asynchronously and blocks once
    at the end; reports (T(N2)-T(N1))/(N2-N1) to cancel fixed dispatch /
    transfer overhead.
    """
    import time

    import jax
    import numpy as np
    from jax.experimental.shard_map import shard_map
    from jax.sharding import Mesh, NamedSharding, PartitionSpec

    import concourse.mybir as mybir
    from concourse.bass2jax import (
        _bass_exec_p,
        install_neuronx_cc_hook,
        partition_id_tensor,
    )

    assert _PROG is not None and LAST_IN_MAPS is not None, "run kernel() first"
    nc = _PROG
    in_maps = LAST_IN_MAPS
    n_cores = 8

    install_neuronx_cc_hook()
    partition_name = nc.partition_id_tensor.name if nc.partition_id_tensor else None
    in_names, out_names, out_avals, zero_outs = [], [], [], []
    for alloc in nc.m.functions[0].allocations:
        if not isinstance(alloc, mybir.MemoryLocationSet):
            continue
        name = alloc.memorylocations[0].name
        if alloc.kind == "ExternalInput":
            if name != partition_name:
                in_names.append(name)
        elif alloc.kind == "ExternalOutput":
            dt = mybir.dt.np(alloc.dtype)
            out_avals.append(jax.core.ShapedArray(tuple(alloc.tensor_shape), dt))
            out_names.append(name)
            zero_outs.append(np.zeros(tuple(alloc.tensor_shape), dt))
    n_params = len(in_names)
    # full operand-name list: inputs, then donated output slots, then
    # partition id — must match run_bass_via_pjrt's convention.
    in_names_full = list(in_names) + list(out_names)
    if partition_name is not None:
        in_names_full.append(partition_name)

    def _body(*args):
        operands = list(args)
        if partition_name is not None:
            operands.append(partition_id_tensor())
        outs = _bass_exec_p.bind(
            *operands,
            out_avals=tuple(out_avals),
            in_names=tuple(in_names_full),
            out_names=tuple(out_names),
            lowering_input_output_aliases=(),
            sim_require_finite=True,
            sim_require_nnan=True,
            nc=nc,
        )
        return tuple(outs)

    devices = jax.devices()[:n_cores]
    mesh = Mesh(np.asarray(devices), ("core",))
    n_outs = len(out_names)
    in_specs = (PartitionSpec("core"),) * (n_params + n_outs)
    out_specs = (PartitionSpec("core"),) * n_outs
    donate = tuple(range(n_params, n_params + n_outs))
    fn = jax.jit(
        shard_map(_body, mesh=mesh, in_specs=in_specs, out_specs=out_specs,
                  check_rep=False),
        donate_argnums=donate,
        keep_unused=True,
    )
    per_core = [[np.asarray(m[name]) for name in in_names] for m in in_maps]
    concat_in = [
        np.concatenate([per_core[c][i] for c in range(n_cores)], axis=0)
        for i in range(n_params)
    ]
    concat_zeros = [
        np.zeros((n_cores * z.shape[0], *z.shape[1:]), z.dtype) for z in zero_outs
    ]
    sh = NamedSharding(mesh, PartitionSpec("core"))
    params_dev = [jax.device_put(a, sh) for a in concat_in]
    z = [jax.device_put(a, sh) for a in concat_zeros]
    # warmup (compile + a few runs); chain outputs into donated slots
    for _ in range(3):
        outs = fn(*params_dev, *z)
        z = list(outs[:n_outs])
    jax.block_until_ready(z)

    def run(n):
        nonlocal z
        t0 = time.perf_counter()
        for _ in range(n):
            outs = fn(*params_dev, *z)
            z = list(outs[:n_outs])
        jax.block_until_ready(z)
        return time.perf_counter() - t0

    n1, n2 = max(5, n_iters // 5), n_iters
    t1 = run(n1)
    t2 = run(n2)
    per_iter = (t2 - t1) / (n2 - n1)
    print(f"benchmark: T({n1})={t1*1e3:.2f}ms T({n2})={t2*1e3:.2f}ms "
          f"slope={per_iter*1e6:.1f}us/iter")
    return per_iter
